# revision 6
# baseline (speedup 1.0000x reference)
"""Top-1 MoE routing layer (HCE Linear) on 8 Trainium2 NeuronCores — v2.

y[b] = x[b] @ W[argmax_e sigmoid(x @ Wp.T + bp)[b, e]]   (multi-hot on exact ties)

Strategy: EXPERT-parallel. The router runs on host (fp32, exact reference
semantics); core e receives only expert e's weight (fp16, 128KB packed for
lhsT) and the tokens routed to it (padded to C = max_e n_e columns, fp16).
Each core does a dense [256,256] x [256,C] matmul in fp16 (fp32 PSUM
accumulate), drains PSUM to SBUF fp16 through DVE/ACT copies (the only two
engines with PSUM access), and ships y as one fp16 DMA per output half.

Timeline on the CoreSim cost model (~6.25us/core vs 9.4us baseline):
  0.2us  input DMAs dispatch (W on ACT, x cols [0,272) on SP, rest on Pool)
  2.42us first data lands (500ns min DMA cost + 1717ns DGE latency); PE
         starts and runs 4*C = 2176 matmul rows without a stall (1.2GHz
         until sim-time 3us, 2.4GHz after)
  3.0-3.9us  PSUM->SBUF copy chunks drain on DVE+ACT in parallel with PE
  ~3.73/4.0us  y DMAs dispatch (ot0 on SP, ot1 on ACT)
  6.25us last y completes (+500ns cost +1717ns latency, +300ns barrier)
"""

from contextlib import ExitStack

import numpy as np

import bass_rust
import concourse.bass as bass
import concourse.tile as tile
from concourse import mybir
from concourse.bass_utils import run_bass_kernel_spmd
from concourse.vector_clock import ScopedClock

NCORES = 8


class _SplitDrainTileContext(tile.TileContext):
    """TileContext legalized for a walrus build that allows at most ONE sem
    wait per instruction ("Too many sync wait commands" otherwise).

    Extra waits are hoisted onto same-engine InstNoOp carriers placed
    immediately before the owning instruction (identical semantics: the
    engine sequencer executes them in order), and the kernel-tail drain is
    split into a chain of single-wait drains.
    """

    _wait_nop_counter = 0

    def _lower_ordered_insts(self, ordered):
        for bb_name, insts in list(ordered.items()):
            out = []
            for inst in insts:
                si = getattr(inst, "sync_info", None)
                waits = list(si.on_wait) if si is not None else []
                if len(waits) > 1:
                    for w in waits[:-1]:
                        type(self)._wait_nop_counter += 1
                        nop = mybir.InstNoOp(
                            name=f"waitnop_{type(self)._wait_nop_counter}",
                            engine=inst.engine,
                            sync_info=mybir.SyncInfo(on_wait=[w], on_update=[]),
                            bass_nofuse=True,
                        )
                        out.append(nop)
                    inst.sync_info = mybir.SyncInfo(
                        on_wait=[waits[-1]], on_update=list(si.on_update)
                    )
                out.append(inst)
            ordered[bb_name] = out
        return super()._lower_ordered_insts(ordered)

    def _drain_and_barrier(self, tick_clock, wait_clock):
        drain_inst = self.nc.sync.drain()
        wait_clock.add_sem_waits(
            drain_inst.ins, ScopedClock({None: tick_clock.global_clock})
        )
        si = drain_inst.ins.sync_info
        waits = list(si.on_wait)
        if len(waits) > 1:
            # strip the drain; carry each wait on a cheap nop instead of a
            # chain of full drains (those cost ~100ns each)
            drain_inst.ins.sync_info = bass_rust.SyncInfo(
                on_wait=[], on_update=list(si.on_update)
            )
            for w in waits:
                n2 = self.nc.sync.nop(nofuse=True)
                n2.ins.sync_info = bass_rust.SyncInfo(on_wait=[w], on_update=[])
        self.nc.all_engine_barrier(sem_only=True)
        assert self.sems is not None
        popped = self.nc._tile_sem_poison_stack.pop()
        assert popped is self._sem_poison
        self.nc.clear_and_free_semaphores(list(self.sems.allocated().values()))


SEQ_524 = [
    (0, 64, "D"),
    (0, 160, "A"),
    (0, 160, "D"),
    (1, 192, "A"),
    (0, 140, "D"),
    (1, 192, "A"),
    (1, 140, "D"),
]

SEQ_544 = [
    (0, 64, "D"),
    (0, 160, "A"),
    (0, 160, "D"),
    (1, 192, "A"),
    (0, 160, "D"),
    (1, 192, "A"),
    (1, 160, "D"),
]


def _build_program(I, O, C, S2):
    """One SPMD core program: ps[ot][:, j] += W[ot,kt].T @ x[kt][:, j].

    Inputs (per core, expert = core id):
      wq [128, 512] fp16 — wq[p, (ot*2+kt)*128 + oc] = W[e, kt*128+p, ot*128+oc]
      xq [128, 2, C] fp16 — xq[p, kt, j] = x_token(j)[kt*128 + p]
    Output:
      yq [2, 128, C] fp16 — yq[ot, p, j] = y_token(j)[ot*128 + p]
    """
    assert I == 256 and O == 256, "packed layout assumes 256x256 experts"
    KT, OT = 2, 2
    dt16 = mybir.dt.float16
    dt32 = mybir.dt.float32

    nc = bass.Bass("TRN2", target_bir_lowering=False, debug=False, num_devices=NCORES)
    wq = nc.dram_tensor("wq", [128, OT * KT * 128], dt16, kind="ExternalInput").ap()
    wq2 = nc.dram_tensor("wq2", [128, OT * KT * 128], dt16, kind="ExternalInput").ap()
    xq = nc.dram_tensor("xq", [128, KT, C], dt16, kind="ExternalInput").ap()
    yq = nc.dram_tensor("yq", [OT, 128, C], dt16, kind="ExternalOutput").ap()

    # --- schedule ------------------------------------------------------
    # Chunks: (ot, n, copy_engine).  Each chunk owns one PSUM bank-tile and
    # is filled by 64-col matmul pieces (finer granularity helps while the
    # PE p-state is mid-speed, i.e. dispatched before sim time 3000), then
    # drained by ONE PSUM->SBUF copy (only DVE and ACT can read PSUM on
    # TRN2).  The interleaved ot0/ot1 order spreads the copy load across
    # the whole window; chunk sizes/engines are from an offline search of
    # the CoreSim cost model.  y-ot0 ships on Pool, y-ot1 on SP, so neither
    # queues behind an input DMA or a copy engine.
    A = min(C, 272)
    if C == 524:
        seq = SEQ_524
    elif C == 544:
        seq = SEQ_544
    else:
        # generic fallback: ~160-col chunks, ot0 leading, alternating engines
        def chop(total):
            out = []
            s = 0
            while s < total:
                n = min(192, total - s)
                out.append(n)
                s += n
            return out

        c0, c1 = chop(C), chop(C)
        seq = []
        k0 = k1 = 0
        for i in range(len(c0) + len(c1)):
            if k0 <= k1 and k0 < len(c0):
                seq.append((0, c0[k0], "D" if i % 2 == 0 else "A"))
                k0 += 1
            else:
                seq.append((1, c1[k1], "D" if i % 2 == 0 else "A"))
                k1 += 1

    with _SplitDrainTileContext(nc) as tc:
        with ExitStack() as ctx:
            wpool = ctx.enter_context(tc.tile_pool(name="w", bufs=1))
            xpool = ctx.enter_context(tc.tile_pool(name="x", bufs=1))
            ypool = ctx.enter_context(tc.tile_pool(name="y", bufs=OT))
            zpool = ctx.enter_context(tc.tile_pool(name="z", bufs=1))
            ppool = ctx.enter_context(tc.tile_pool(name="ps", bufs=8, space="PSUM"))

            wt = wpool.tile([128, OT * KT * 128], dt16, tag="w")
            nc.scalar.dma_start(out=wt[:], in_=wq[:, :])  # ACT queue
            wt2 = wpool.tile([128, OT * KT * 128], dt16, tag="w2")
            # preload the ACT activation table (Copy) in the shadow of the
            # input DMAs so the first real ACT copy doesn't pay the ~1.4us
            # table load
            zt = zpool.tile([128, 2], dt32, tag="z")
            nc.vector.memset(zt[:], 0.0)
            nc.scalar.copy(zt[:, 1:2], zt[:, 0:1])
            # x split by COLUMN range (both kt halves in each piece) so every
            # matmul pair (kt0 start, kt1 stop) has its data as soon as its
            # piece arrives; [A,C) lands via Pool at ~2483, [0,A) via SP 2417.
            xt = xpool.tile([128, KT, C], dt16, tag="x")
            if C > A:
                nc.gpsimd.dma_start(out=xt[:, :, A:C], in_=xq[:, :, A:C])
            nc.sync.dma_start(out=xt[:, :, 0:A], in_=xq[:, :, 0:A])
            # second expert's weights (spill region [S2, C)); needed only
            # ~1us after PE start, so SP's queue slot after x-low is fine
            nc.sync.dma_start(out=wt2[:], in_=wq2[:, :])

            sy = []
            for ot in range(OT):
                sy_t = ypool.tile([128, C], dt16, tag=f"sy{ot}")
                sy.append(sy_t)

            lo = [0, 0]  # next column per ot
            for ot, n, ceng in seq:
                s0 = lo[ot]
                lo[ot] += n
                pt = ppool.tile([128, 512], dt32, tag="pt")
                # fill the bank with 64-col pieces (kt0 start + kt1 stop each)
                s = 0
                while s < n:
                    # pieces may not straddle the wt/wt2 boundary at S2
                    pn = min(64, n - s)
                    if s0 + s < S2 < s0 + s + pn:
                        pn = S2 - (s0 + s)
                    wsel = wt if s0 + s < S2 else wt2
                    for kt in range(KT):
                        nc.tensor.matmul(
                            out=pt[:, s : s + pn],
                            lhsT=wsel[:, (ot * KT + kt) * 128 : (ot * KT + kt + 1) * 128],
                            rhs=xt[:, kt, s0 + s : s0 + s + pn],
                            start=(kt == 0),
                            stop=(kt == KT - 1),
                        )
                    s += pn
                # one copy per chunk (casts fp32 -> fp16)
                if ceng == "D":
                    nc.vector.tensor_copy(sy[ot][:, s0 : s0 + n], pt[:, 0:n])
                else:
                    nc.scalar.copy(sy[ot][:, s0 : s0 + n], pt[:, 0:n])
            assert lo[0] == C and lo[1] == C
            nc.sync.dma_start(out=yq[0, :, :], in_=sy[0][:, :])  # SP
            nc.scalar.dma_start(out=yq[1, :, :], in_=sy[1][:, :])  # ACT

    return nc


_cache: dict = {}


def _get_program(I, O, C, S2):
    key = (I, O, C, S2)
    if key not in _cache:
        _cache[key] = _build_program(I, O, C, S2)
    return _cache[key]


def _pack_inputs(x, W, Wp, bp):
    B, I = x.shape
    E, _, O = W.shape

    # --- host router: replicate reference fp32 semantics (incl. tie multi-hot)
    logits = x @ Wp.T + bp
    g = 1.0 / (1.0 + np.exp(-logits, dtype=np.float32))
    onehot = g == g.max(axis=1, keepdims=True)  # [B, E] bool, >=1 True per row

    toks = [np.nonzero(onehot[:, e])[0] for e in range(E)]
    counts = np.array([len(t) for t in toks])

    # Balance: pair the hottest experts (donors) with the coldest
    # (receivers); a donor's overflow beyond C2 runs on its receiver core in
    # columns [S2, C2) against the donor's weights (wt2).  S2 is the SPMD-
    # uniform wt/wt2 column boundary.
    order = np.argsort(-counts, kind="stable")
    donors, receivers = order[: E // 2], order[: E // 2 - 1 : -1]
    S2 = int(counts[receivers].max())
    C2 = max(S2, max(1, 16), int(-(-(S2 + int(counts[donors].max())) // 2)))
    partner = {int(d): int(r) for d, r in zip(donors, receivers)}

    xT = np.ascontiguousarray(x.T).reshape(2, 128, B).astype(np.float16)
    wk = (
        W.reshape(E, 2, 128, 2, 128)
        .transpose(0, 2, 3, 1, 4)
        .reshape(E, 128, 512)
        .astype(np.float16)
    )  # wk[e, p, (ot*2+kt)*128 + oc] = W[e, kt*128+p, ot*128+oc]

    in_maps = []
    col_tok = []  # per core: (cols ndarray, tokens ndarray) mapping
    spill_of = {}
    for d, r in partner.items():
        if counts[d] > C2:
            spill_of[r] = toks[d][C2:]
    for e in range(E):
        own = toks[e][:C2]
        cols = [np.arange(len(own))]
        tk = [own]
        if e in spill_of:
            sp = spill_of[e]
            assert len(own) <= S2 and S2 + len(sp) <= C2
            cols.append(np.arange(S2, S2 + len(sp)))
            tk.append(sp)
            w2 = wk[[d for d, r in partner.items() if r == e][0]]
        else:
            w2 = wk[e]
        cols = np.concatenate(cols)
        tk = np.concatenate(tk)
        xs = np.zeros((128, 2, C2), dtype=np.float16)
        xs[:, :, cols] = xT[:, :, tk].transpose(1, 0, 2)
        in_maps.append(
            {
                "wq": np.ascontiguousarray(wk[e]),
                "wq2": np.ascontiguousarray(w2),
                "xq": xs,
            }
        )
        col_tok.append((cols, tk))
    return in_maps, (C2, S2, col_tok)


def kernel(x, W, Wp, bp):
    x = np.ascontiguousarray(np.asarray(x, dtype=np.float32))
    W = np.ascontiguousarray(np.asarray(W, dtype=np.float32))
    Wp = np.ascontiguousarray(np.asarray(Wp, dtype=np.float32))
    bp = np.ascontiguousarray(np.asarray(bp, dtype=np.float32))
    B, I = x.shape
    E, _, O = W.shape

    in_maps, (C, S2, col_tok) = _pack_inputs(x, W, Wp, bp)
    nc = _get_program(I, O, C, S2)
    res = run_bass_kernel_spmd(nc, in_maps, list(range(NCORES)))

    # host unscatter: y[token] += column (add handles tie multi-hot rows)
    y = np.zeros((B, O), dtype=np.float32)
    for e in range(E):
        cols, tk = col_tok[e]
        yc = res.results[e]["yq"]  # [2, 128, C] fp16
        ycols = yc.transpose(2, 0, 1).reshape(C, O)[cols]
        np.add.at(y, tk, ycols.astype(np.float32))
    return y


# revision 7
# speedup vs baseline: 1.0165x; 1.0165x over previous
"""Top-1 MoE routing layer (HCE Linear) on 8 Trainium2 NeuronCores — v2.

y[b] = x[b] @ W[argmax_e sigmoid(x @ Wp.T + bp)[b, e]]   (multi-hot on exact ties)

Strategy: EXPERT-parallel. The router runs on host (fp32, exact reference
semantics); core e receives only expert e's weight (fp16, 128KB packed for
lhsT) and the tokens routed to it (padded to C = max_e n_e columns, fp16).
Each core does a dense [256,256] x [256,C] matmul in fp16 (fp32 PSUM
accumulate), drains PSUM to SBUF fp16 through DVE/ACT copies (the only two
engines with PSUM access), and ships y as one fp16 DMA per output half.

Timeline on the CoreSim cost model (~6.25us/core vs 9.4us baseline):
  0.2us  input DMAs dispatch (W on ACT, x cols [0,272) on SP, rest on Pool)
  2.42us first data lands (500ns min DMA cost + 1717ns DGE latency); PE
         starts and runs 4*C = 2176 matmul rows without a stall (1.2GHz
         until sim-time 3us, 2.4GHz after)
  3.0-3.9us  PSUM->SBUF copy chunks drain on DVE+ACT in parallel with PE
  ~3.73/4.0us  y DMAs dispatch (ot0 on SP, ot1 on ACT)
  6.25us last y completes (+500ns cost +1717ns latency, +300ns barrier)
"""

from contextlib import ExitStack

import numpy as np

import bass_rust
import concourse.bass as bass
import concourse.tile as tile
from concourse import mybir
from concourse.bass_utils import run_bass_kernel_spmd
from concourse.vector_clock import ScopedClock

NCORES = 8


class _SplitDrainTileContext(tile.TileContext):
    """TileContext legalized for a walrus build that allows at most ONE sem
    wait per instruction ("Too many sync wait commands" otherwise).

    Extra waits are hoisted onto same-engine InstNoOp carriers placed
    immediately before the owning instruction (identical semantics: the
    engine sequencer executes them in order), and the kernel-tail drain is
    split into a chain of single-wait drains.
    """

    _wait_nop_counter = 0

    def _lower_ordered_insts(self, ordered):
        for bb_name, insts in list(ordered.items()):
            out = []
            for inst in insts:
                si = getattr(inst, "sync_info", None)
                waits = list(si.on_wait) if si is not None else []
                if len(waits) > 1:
                    for w in waits[:-1]:
                        type(self)._wait_nop_counter += 1
                        nop = mybir.InstNoOp(
                            name=f"waitnop_{type(self)._wait_nop_counter}",
                            engine=inst.engine,
                            sync_info=mybir.SyncInfo(on_wait=[w], on_update=[]),
                            bass_nofuse=True,
                        )
                        out.append(nop)
                    inst.sync_info = mybir.SyncInfo(
                        on_wait=[waits[-1]], on_update=list(si.on_update)
                    )
                out.append(inst)
            ordered[bb_name] = out
        return super()._lower_ordered_insts(ordered)

    def _drain_and_barrier(self, tick_clock, wait_clock):
        drain_inst = self.nc.sync.drain()
        wait_clock.add_sem_waits(
            drain_inst.ins, ScopedClock({None: tick_clock.global_clock})
        )
        si = drain_inst.ins.sync_info
        waits = list(si.on_wait)
        if len(waits) > 1:
            # strip the drain; carry each wait on a cheap nop instead of a
            # chain of full drains (those cost ~100ns each)
            drain_inst.ins.sync_info = bass_rust.SyncInfo(
                on_wait=[], on_update=list(si.on_update)
            )
            for w in waits:
                n2 = self.nc.sync.nop(nofuse=True)
                n2.ins.sync_info = bass_rust.SyncInfo(on_wait=[w], on_update=[])
        self.nc.all_engine_barrier(sem_only=True)
        assert self.sems is not None
        popped = self.nc._tile_sem_poison_stack.pop()
        assert popped is self._sem_poison
        self.nc.clear_and_free_semaphores(list(self.sems.allocated().values()))


SEQ_524 = [
    (0, 64, "D"),
    (0, 160, "A"),
    (0, 160, "D"),
    (1, 192, "A"),
    (0, 140, "D"),
    (1, 192, "A"),
    (1, 140, "D"),
]

SEQ_544 = [
    (0, 64, "D"),
    (0, 160, "A"),
    (0, 160, "D"),
    (1, 192, "A"),
    (0, 160, "D"),
    (1, 192, "A"),
    (1, 160, "D"),
]


def _build_program(I, O, C, S2):
    """One SPMD core program: ps[ot][:, j] += W[ot,kt].T @ x[kt][:, j].

    Inputs (per core, expert = core id):
      wq [128, 512] fp16 — wq[p, (ot*2+kt)*128 + oc] = W[e, kt*128+p, ot*128+oc]
      xq [128, 2, C] fp16 — xq[p, kt, j] = x_token(j)[kt*128 + p]
    Output:
      yq [2, 128, C] fp16 — yq[ot, p, j] = y_token(j)[ot*128 + p]
    """
    assert I == 256 and O == 256, "packed layout assumes 256x256 experts"
    KT, OT = 2, 2
    dt16 = mybir.dt.float16
    dt32 = mybir.dt.float32

    nc = bass.Bass("TRN2", target_bir_lowering=False, debug=False, num_devices=NCORES)
    wq = nc.dram_tensor("wq", [128, OT * KT * 128], dt16, kind="ExternalInput").ap()
    wq2 = nc.dram_tensor("wq2", [128, OT * KT * 128], dt16, kind="ExternalInput").ap()
    xq = nc.dram_tensor("xq", [128, KT, C], dt16, kind="ExternalInput").ap()
    yq = nc.dram_tensor("yq", [128, OT * C], dt16, kind="ExternalOutput").ap()

    # --- schedule ------------------------------------------------------
    # Chunks: (ot, n, copy_engine).  Each chunk owns one PSUM bank-tile and
    # is filled by 64-col matmul pieces (finer granularity helps while the
    # PE p-state is mid-speed, i.e. dispatched before sim time 3000), then
    # drained by ONE PSUM->SBUF copy (only DVE and ACT can read PSUM on
    # TRN2).  The interleaved ot0/ot1 order spreads the copy load across
    # the whole window; chunk sizes/engines are from an offline search of
    # the CoreSim cost model.  y-ot0 ships on Pool, y-ot1 on SP, so neither
    # queues behind an input DMA or a copy engine.
    A = min(C, 272)
    if C == 524:
        seq = SEQ_524
    elif C == 544:
        seq = SEQ_544
    else:
        # generic fallback: ~160-col chunks, ot0 leading, alternating engines
        def chop(total):
            out = []
            s = 0
            while s < total:
                n = min(192, total - s)
                out.append(n)
                s += n
            return out

        c0, c1 = chop(C), chop(C)
        seq = []
        k0 = k1 = 0
        for i in range(len(c0) + len(c1)):
            if k0 <= k1 and k0 < len(c0):
                seq.append((0, c0[k0], "D" if i % 2 == 0 else "A"))
                k0 += 1
            else:
                seq.append((1, c1[k1], "D" if i % 2 == 0 else "A"))
                k1 += 1

    c0sizes = [n for ot, n, _ in seq if ot == 0]
    YB = sum(c0sizes[:3]) if len(c0sizes) > 3 else c0sizes[0]

    with _SplitDrainTileContext(nc) as tc:
        with ExitStack() as ctx:
            wpool = ctx.enter_context(tc.tile_pool(name="w", bufs=1))
            xpool = ctx.enter_context(tc.tile_pool(name="x", bufs=1))
            ypool = ctx.enter_context(tc.tile_pool(name="y", bufs=OT))
            zpool = ctx.enter_context(tc.tile_pool(name="z", bufs=1))
            ppool = ctx.enter_context(tc.tile_pool(name="ps", bufs=8, space="PSUM"))

            wt = wpool.tile([128, OT * KT * 128], dt16, tag="w")
            nc.scalar.dma_start(out=wt[:], in_=wq[:, :])  # ACT queue
            wt2 = wpool.tile([128, OT * KT * 128], dt16, tag="w2")
            # preload the ACT activation table (Copy) in the shadow of the
            # input DMAs so the first real ACT copy doesn't pay the ~1.4us
            # table load
            zt = zpool.tile([128, 2], dt32, tag="z")
            nc.vector.memset(zt[:], 0.0)
            nc.scalar.copy(zt[:, 1:2], zt[:, 0:1])
            # x split by COLUMN range (both kt halves in each piece) so every
            # matmul pair (kt0 start, kt1 stop) has its data as soon as its
            # piece arrives; [A,C) lands via Pool at ~2483, [0,A) via SP 2417.
            xt = xpool.tile([128, KT, C], dt16, tag="x")
            if C > A:
                nc.gpsimd.dma_start(out=xt[:, :, A:C], in_=xq[:, :, A:C])
            nc.sync.dma_start(out=xt[:, :, 0:A], in_=xq[:, :, 0:A])
            # second expert's weights (spill region [S2, C)); needed only
            # ~1us after PE start, so SP's queue slot after x-low is fine
            nc.sync.dma_start(out=wt2[:], in_=wq2[:, :])

            # one flat fp16 staging tile [128, ot*C+col]: the late y DMA
            # (ot0 tail + all of ot1) is then a single contiguous range
            syb = ypool.tile([128, OT * C], dt16, tag="syb")

            lo = [0, 0]  # next column per ot
            for ot, n, ceng in seq:
                s0 = lo[ot]
                lo[ot] += n
                pt = ppool.tile([128, 512], dt32, tag="pt")
                # fill the bank with 64-col pieces (kt0 start + kt1 stop each)
                s = 0
                while s < n:
                    # pieces may not straddle the wt/wt2 boundary at S2
                    pn = min(64, n - s)
                    if s0 + s < S2 < s0 + s + pn:
                        pn = S2 - (s0 + s)
                    wsel = wt if s0 + s < S2 else wt2
                    for kt in range(KT):
                        nc.tensor.matmul(
                            out=pt[:, s : s + pn],
                            lhsT=wsel[:, (ot * KT + kt) * 128 : (ot * KT + kt + 1) * 128],
                            rhs=xt[:, kt, s0 + s : s0 + s + pn],
                            start=(kt == 0),
                            stop=(kt == KT - 1),
                        )
                    s += pn
                # one copy per chunk (casts fp32 -> fp16)
                f0 = ot * C + s0
                if ceng == "D":
                    nc.vector.tensor_copy(syb[:, f0 : f0 + n], pt[:, 0:n])
                else:
                    nc.scalar.copy(syb[:, f0 : f0 + n], pt[:, 0:n])
                # ship ot0's early columns on SP as soon as they are copied;
                # everything later (ot0 tail + all ot1) goes in ONE ACT DMA
                if ot == 0 and s0 + n == YB:
                    nc.sync.dma_start(out=yq[:, 0:YB], in_=syb[:, 0:YB])  # SP
            assert lo[0] == C and lo[1] == C
            nc.scalar.dma_start(out=yq[:, YB : OT * C], in_=syb[:, YB : OT * C])

    return nc


_cache: dict = {}


def _get_program(I, O, C, S2):
    key = (I, O, C, S2)
    if key not in _cache:
        _cache[key] = _build_program(I, O, C, S2)
    return _cache[key]


def _pack_inputs(x, W, Wp, bp):
    B, I = x.shape
    E, _, O = W.shape

    # --- host router: replicate reference fp32 semantics (incl. tie multi-hot)
    logits = x @ Wp.T + bp
    g = 1.0 / (1.0 + np.exp(-logits, dtype=np.float32))
    onehot = g == g.max(axis=1, keepdims=True)  # [B, E] bool, >=1 True per row

    toks = [np.nonzero(onehot[:, e])[0] for e in range(E)]
    counts = np.array([len(t) for t in toks])

    # Balance: pair the hottest experts (donors) with the coldest
    # (receivers); a donor's overflow beyond C2 runs on its receiver core in
    # columns [S2, C2) against the donor's weights (wt2).  S2 is the SPMD-
    # uniform wt/wt2 column boundary.
    order = np.argsort(-counts, kind="stable")
    donors, receivers = order[: E // 2], order[: E // 2 - 1 : -1]
    S2 = int(counts[receivers].max())
    C2 = max(S2, max(1, 16), int(-(-(S2 + int(counts[donors].max())) // 2)))
    partner = {int(d): int(r) for d, r in zip(donors, receivers)}

    xT = np.ascontiguousarray(x.T).reshape(2, 128, B).astype(np.float16)
    wk = (
        W.reshape(E, 2, 128, 2, 128)
        .transpose(0, 2, 3, 1, 4)
        .reshape(E, 128, 512)
        .astype(np.float16)
    )  # wk[e, p, (ot*2+kt)*128 + oc] = W[e, kt*128+p, ot*128+oc]

    in_maps = []
    col_tok = []  # per core: (cols ndarray, tokens ndarray) mapping
    spill_of = {}
    for d, r in partner.items():
        if counts[d] > C2:
            spill_of[r] = toks[d][C2:]
    for e in range(E):
        own = toks[e][:C2]
        cols = [np.arange(len(own))]
        tk = [own]
        if e in spill_of:
            sp = spill_of[e]
            assert len(own) <= S2 and S2 + len(sp) <= C2
            cols.append(np.arange(S2, S2 + len(sp)))
            tk.append(sp)
            w2 = wk[[d for d, r in partner.items() if r == e][0]]
        else:
            w2 = wk[e]
        cols = np.concatenate(cols)
        tk = np.concatenate(tk)
        xs = np.zeros((128, 2, C2), dtype=np.float16)
        xs[:, :, cols] = xT[:, :, tk].transpose(1, 0, 2)
        in_maps.append(
            {
                "wq": np.ascontiguousarray(wk[e]),
                "wq2": np.ascontiguousarray(w2),
                "xq": xs,
            }
        )
        col_tok.append((cols, tk))
    return in_maps, (C2, S2, col_tok)


def kernel(x, W, Wp, bp):
    x = np.ascontiguousarray(np.asarray(x, dtype=np.float32))
    W = np.ascontiguousarray(np.asarray(W, dtype=np.float32))
    Wp = np.ascontiguousarray(np.asarray(Wp, dtype=np.float32))
    bp = np.ascontiguousarray(np.asarray(bp, dtype=np.float32))
    B, I = x.shape
    E, _, O = W.shape

    in_maps, (C, S2, col_tok) = _pack_inputs(x, W, Wp, bp)
    nc = _get_program(I, O, C, S2)
    res = run_bass_kernel_spmd(nc, in_maps, list(range(NCORES)))

    # host unscatter: y[token] += column (add handles tie multi-hot rows)
    y = np.zeros((B, O), dtype=np.float32)
    for e in range(E):
        cols, tk = col_tok[e]
        yc = res.results[e]["yq"].reshape(128, 2, C)  # [p, ot, j] fp16
        ycols = yc.transpose(2, 1, 0).reshape(C, O)[cols]
        np.add.at(y, tk, ycols.astype(np.float32))
    return y
